# revision 7
# baseline (speedup 1.0000x reference)
"""BankedLinear (MoE-style banked linear) Trainium2 Bass kernel.

Math: out[n] = sum_k bank_weights[n,k] * (tensor[n] @ W[sel[n,k]] + bias[sel[n,k]])
Shapes: tensor [8192,128] f32, bank_weights [8192,2] f32, bank_selections [8192,2] int,
        weights [64,128,128] f32, bias [64,128] f32 -> out [8192,128] f32.

Strategy (data parallel over tokens, weights replicated, fp16 compute):
  - 8 cores x 1024 tokens, greedily assigned so per-bank pair counts are
    balanced across cores (SPMD: one program, shared per-bank capacity plan).
  - Two sorted passes per core. Pass A processes each token's k=0 pair with
    tokens sorted by sel[:,0]; pass B processes k=1 pairs sorted by sel[:,1].
    The host pre-builds x^T column tiles in each sorted order with the
    bank_weight folded in (column s = bw[pair s] * x[token s], fp16), so the
    device computes psum[o, s] = sum_i W_b[i,o]*xbw[i, s] + bias_b[o]*bw[s]
    with two chained matmuls per bank (the bias term is a rank-1 matmul with
    a one-partition lhsT). Pad columns are zero.
  - psum -> fp16 SBUF evict, PE-transpose back to row layout, evict to f32
    rows. Pass A rows land in slot order = output order: one contiguous DMA
    to out. Pass B rows are combined with a single SWDGE dma_scatter_add
    (out[idx[j]] += rows1[j]) whose per-core index tensor maps each k=1 slot
    to its token's k=0 slot. Pad rows are exactly zero and target row 0.
  - Host unshards: out[assign[c][i]] = dev_out[c][pos0[c][i]].
"""

import numpy as np

N, K, IN, OUT, NUM_BANKS = 8192, 2, 128, 128, 64
NCORES = 8
NLOC = N // NCORES
P = 128
PSUM_FREE = 512


def _routing_plan(sel_all):
    """Greedy token->core assignment balancing per-bank counts for both the
    k=0 and k=1 pair distributions. Returns (assign, caps0, offs0, Cap0,
    caps1, offs1, Cap1) where caps are shared by all cores (SPMD)."""
    sel_all = np.asarray(sel_all).astype(np.int64)
    g0 = np.bincount(sel_all[:, 0], minlength=NUM_BANKS)
    g1 = np.bincount(sel_all[:, 1], minlength=NUM_BANKS)
    ideal0 = (g0 + NCORES - 1) // NCORES
    ideal1 = (g1 + NCORES - 1) // NCORES
    c0 = np.zeros((NCORES, NUM_BANKS), dtype=np.int64)
    c1 = np.zeros((NCORES, NUM_BANKS), dtype=np.int64)
    fill = np.zeros(NCORES, dtype=np.int64)
    assign_lists = [[] for _ in range(NCORES)]
    for n in range(N):
        b0, b1 = int(sel_all[n, 0]), int(sel_all[n, 1])
        best, best_key = -1, None
        for c in range(NCORES):
            if fill[c] >= NLOC:
                continue
            over = max(0, c0[c, b0] + 1 - ideal0[b0]) + \
                max(0, c1[c, b1] + 1 - ideal1[b1])
            key = (over, c0[c, b0] + c1[c, b1], fill[c])
            if best < 0 or key < best_key:
                best, best_key = c, key
        c0[best, b0] += 1
        c1[best, b1] += 1
        fill[best] += 1
        assign_lists[best].append(n)
    assign = np.array(assign_lists, dtype=np.int64)

    def cap_plan(counts):
        caps = counts.max(axis=0).astype(np.int64)
        pad = (-int(caps.sum())) % P
        for i in range(pad):
            caps[i % NUM_BANKS] += 1
        offs = np.concatenate([[0], np.cumsum(caps)[:-1]]).astype(np.int64)
        return caps, offs, int(caps.sum())

    caps0, offs0, Cap0 = cap_plan(c0)
    caps1, offs1, Cap1 = cap_plan(c1)
    return assign, caps0, offs0, Cap0, caps1, offs1, Cap1


def _wrap_idx(flat_idx):
    """Wrap a flat int16 index list into the [128, n//16] SWDGE layout."""
    n = flat_idx.shape[0]
    assert n % 16 == 0
    w = flat_idx.reshape(n // 16, 16).T.astype(np.int16)
    return np.tile(w, (8, 1))


def _segments(caps, offs):
    """Per-bank psum column segments split at PSUM_FREE boundaries.
    Returns list of (bank, col_start, width) with width>0."""
    segs = []
    for b in range(NUM_BANKS):
        cb, ob = int(caps[b]), int(offs[b])
        while cb > 0:
            room = PSUM_FREE - (ob % PSUM_FREE)
            w = min(cb, room)
            segs.append((b, ob, w))
            ob += w
            cb -= w
    return segs


def _build_program(caps0, offs0, Cap0, caps1, offs1, Cap1):
    import concourse.bacc as bacc
    import concourse.tile as tile
    from concourse import mybir, library_config
    from concourse.masks import make_identity
    from concourse.tile import add_dep_helper

    f32 = mybir.dt.float32
    f16 = mybir.dt.float16
    i16 = mybir.dt.int16

    nblk0, nblk1 = Cap0 // P, Cap1 // P
    nch0 = (Cap0 + PSUM_FREE - 1) // PSUM_FREE
    nch1 = (Cap1 + PSUM_FREE - 1) // PSUM_FREE

    nc = bacc.Bacc(None, target_bir_lowering=False, debug=False)

    x0_d = nc.declare_dram_parameter("x0", [P, Cap0], f16, isOutput=False)
    x1_d = nc.declare_dram_parameter("x1", [P, Cap1], f16, isOutput=False)
    w_d = nc.declare_dram_parameter("wts", [P, NUM_BANKS * OUT], f16,
                                    isOutput=False)
    bias_d = nc.declare_dram_parameter("biasb", [1, NUM_BANKS * OUT], f16,
                                       isOutput=False)
    bw0_d = nc.declare_dram_parameter("bw0", [1, Cap0], f16, isOutput=False)
    bw1_d = nc.declare_dram_parameter("bw1", [1, Cap1], f16, isOutput=False)
    idx_d = nc.declare_dram_parameter("sidx", [P, Cap1 // 16], i16,
                                      isOutput=False)
    out_d = nc.declare_dram_parameter("out", [Cap0, OUT], f32, isOutput=True)

    segs0 = _segments(caps0, offs0)
    segs1 = _segments(caps1, offs1)

    # weight DMA split across three queues, in bank order
    wsplit = (22, 22, 20)
    wb = [0, wsplit[0], wsplit[0] + wsplit[1], NUM_BANKS]

    with tile.TileContext(nc) as tc:
        with (
            tc.tile_pool(name="const", bufs=1) as cpool,
            tc.tile_pool(name="big", bufs=1) as bigpool,
            tc.tile_pool(name="psA", bufs=1, space="PSUM") as psA,
            tc.tile_pool(name="psB", bufs=1, space="PSUM") as psB,
            tc.tile_pool(name="psT", bufs=2, space="PSUM") as psT,
        ):
            ident = cpool.tile([P, P], f16)
            make_identity(nc, ident[:])
            libload = nc.gpsimd.load_library(library_config.mlp)

            # input DMAs
            xs0 = cpool.tile([P, Cap0], f16)
            nc.sync.dma_start(out=xs0[:], in_=x0_d.ap())
            xs1 = cpool.tile([P, Cap1], f16)
            nc.sync.dma_start(out=xs1[:], in_=x1_d.ap())

            w_sb = cpool.tile([P, NUM_BANKS * OUT], f16)
            for si, eng in zip(range(3), (nc.scalar, nc.sync, nc.gpsimd)):
                nc_lo, nc_hi = wb[si] * OUT, wb[si + 1] * OUT
                eng.dma_start(out=w_sb[:, nc_lo:nc_hi],
                              in_=w_d[:, nc_lo:nc_hi])

            bias_sb = cpool.tile([1, NUM_BANKS * OUT], f16)
            nc.sync.dma_start(out=bias_sb[:], in_=bias_d.ap())
            bw0_sb = cpool.tile([1, Cap0], f16)
            nc.sync.dma_start(out=bw0_sb[:], in_=bw0_d.ap())
            bw1_sb = cpool.tile([1, Cap1], f16)
            nc.sync.dma_start(out=bw1_sb[:], in_=bw1_d.ap())
            idx_sb = cpool.tile([P, Cap1 // 16], i16)
            nc.sync.dma_start(out=idx_sb[:], in_=idx_d.ap())

            # per-bank matmuls into chunked psum tiles (both passes)
            pA, pB = [], []
            for i in range(nch0):
                pt = psA.tile([P, min(PSUM_FREE, Cap0 - i * PSUM_FREE)], f32,
                              tag=f"pA{i}", name=f"pA{i}")
                pA.append(pt)
            for i in range(nch1):
                pt = psB.tile([P, min(PSUM_FREE, Cap1 - i * PSUM_FREE)], f32,
                              tag=f"pB{i}", name=f"pB{i}")
                pB.append(pt)

            def do_pass(segs, ptiles, xs, bw_sb):
                for (b, ob, wseg) in segs:
                    ch, co = ob // PSUM_FREE, ob % PSUM_FREE
                    lhs = w_sb[:, b * OUT:(b + 1) * OUT]
                    nc.tensor.matmul(out=ptiles[ch][:, co:co + wseg],
                                     lhsT=lhs, rhs=xs[:, ob:ob + wseg],
                                     start=True, stop=False)
                    nc.tensor.matmul(out=ptiles[ch][:, co:co + wseg],
                                     lhsT=bias_sb[:, b * OUT:(b + 1) * OUT],
                                     rhs=bw_sb[:, ob:ob + wseg],
                                     start=False, stop=True)

            do_pass(segs0, pA, xs0, bw0_sb)
            do_pass(segs1, pB, xs1, bw1_sb)

            # evict psum chunks -> fp16, transpose -> f32 rows
            y0T = bigpool.tile([P, Cap0], f16, tag="y0T")
            y1T = bigpool.tile([P, Cap1], f16, tag="y1T")
            rows0 = bigpool.tile([P, nblk0, OUT], f32, tag="rows0")
            rows1 = bigpool.tile([P, nblk1, OUT], f32, tag="rows1")

            flip = 0
            for (yT, rows, ptiles, nblk, Cap) in (
                    (y0T, rows0, pA, nblk0, Cap0),
                    (y1T, rows1, pB, nblk1, Cap1)):
                for ch in range(len(ptiles)):
                    lo = ch * PSUM_FREE
                    wch = min(PSUM_FREE, Cap - lo)
                    h = wch // 2
                    eng0 = nc.vector.tensor_copy if flip == 0 else nc.scalar.copy
                    eng1 = nc.scalar.copy if flip == 0 else nc.vector.tensor_copy
                    if h > 0:
                        eng0(yT[:, lo:lo + h], ptiles[ch][:, :h])
                        eng1(yT[:, lo + h:lo + wch], ptiles[ch][:, h:wch])
                    else:
                        eng0(yT[:, lo:lo + wch], ptiles[ch][:, :wch])
                    flip ^= 1
                for t in range(nblk):
                    ptt = psT.tile([P, P], f16, tag="ptt")
                    nc.tensor.transpose(out=ptt[:], in_=yT[:, t * P:(t + 1) * P],
                                        identity=ident[:])
                    if t % 2 == 0:
                        nc.vector.tensor_copy(rows[:, t, :], ptt[:])
                    else:
                        nc.scalar.copy(rows[:, t, :], ptt[:])

            # pass A rows -> contiguous out; pass B rows -> scatter-add
            hb = nblk0 // 2
            nc.sync.dma_start(
                out=out_d[:hb * P].rearrange("(t p) o -> p t o", p=P),
                in_=rows0[:, :hb, :])
            nc.sync.dma_start(
                out=out_d[hb * P:].rearrange("(t p) o -> p t o", p=P),
                in_=rows0[:, hb:, :])
            sa = nc.gpsimd.dma_scatter_add(
                out_ap=out_d.ap(),
                in_ap=rows1[:, :, :],
                idxs_ap=idx_sb[:],
                num_idxs=Cap1,
                num_idxs_reg=Cap1,
                elem_size=OUT,
                single_packet=Cap1 <= 1024,
            )
            add_dep_helper(sa.ins, libload.ins, sync=False,
                           reason="scatter-add needs mlp gpsimd library")

    return nc


def _make_in_maps(tensor, bank_weights, bank_selections, weights, bias,
                  assign, caps0, offs0, Cap0, caps1, offs1, Cap1):
    tensor = np.ascontiguousarray(tensor, dtype=np.float32)
    bank_weights = np.ascontiguousarray(bank_weights, dtype=np.float32)
    sel_all = np.asarray(bank_selections).astype(np.int64)
    w16 = np.ascontiguousarray(
        np.asarray(weights, dtype=np.float32).transpose(1, 0, 2)
        .reshape(IN, NUM_BANKS * OUT)).astype(np.float16)
    bias16 = np.ascontiguousarray(bias, dtype=np.float32).astype(
        np.float16).reshape(1, NUM_BANKS * OUT)

    in_maps = []
    pos0_all = []
    for c in range(NCORES):
        toks = assign[c]
        sel = sel_all[toks]                      # [NLOC, K]
        bw = bank_weights[toks]                  # [NLOC, K]
        x = tensor[toks]                         # [NLOC, IN]

        # slot layout per pass: tokens sorted by bank, at offs[b] + rank
        def lay(k, caps, offs, Cap):
            slot = np.zeros(NLOC, dtype=np.int64)
            fillb = offs.copy()
            for i in range(NLOC):
                b = sel[i, k]
                slot[i] = fillb[b]
                fillb[b] += 1
            xbw = np.zeros((Cap, IN), dtype=np.float32)
            xbw[slot] = x * bw[:, k:k + 1]
            bwrow = np.zeros((1, Cap), dtype=np.float32)
            bwrow[0, slot] = bw[:, k]
            return slot, np.ascontiguousarray(xbw.T).astype(np.float16), \
                bwrow.astype(np.float16)

        slot0, x0, bw0row = lay(0, caps0, offs0, Cap0)
        slot1, x1, bw1row = lay(1, caps1, offs1, Cap1)
        pos0_all.append(slot0)

        # scatter indices: k=1 slot j -> that token's k=0 slot. Pad rows are
        # all-zero but their scatter-add is a DRAM read-modify-write that can
        # race a real add to the same row on hardware — point them at pass-A
        # pad slots (rows no token reads) instead of a shared real row.
        pad0 = np.setdiff1d(np.arange(Cap0, dtype=np.int64), slot0)
        assert pad0.size > 0
        reps = (Cap1 + pad0.size - 1) // pad0.size
        sidx = np.tile(pad0, reps)[:Cap1]
        sidx[slot1] = slot0
        in_maps.append({
            "x0": x0,
            "x1": x1,
            "wts": w16,
            "biasb": bias16,
            "bw0": bw0row,
            "bw1": bw1row,
            "sidx": _wrap_idx(sidx.astype(np.int16)),
        })
    return in_maps, pos0_all


def kernel(tensor, bank_weights, bank_selections, weights, bias):
    tensor = np.asarray(tensor)
    bank_weights = np.asarray(bank_weights)
    bank_selections = np.asarray(bank_selections)
    weights = np.asarray(weights)
    bias = np.asarray(bias)

    assign, caps0, offs0, Cap0, caps1, offs1, Cap1 = \
        _routing_plan(bank_selections)
    nc = _build_program(caps0, offs0, Cap0, caps1, offs1, Cap1)
    in_maps, pos0_all = _make_in_maps(
        tensor, bank_weights, bank_selections, weights, bias,
        assign, caps0, offs0, Cap0, caps1, offs1, Cap1)

    nc.finalize()
    from concourse.bass_utils import run_bass_kernel_spmd
    try:
        res = run_bass_kernel_spmd(nc, in_maps, list(range(NCORES)))
    except Exception:
        # one retry: a previous crashed session can leave the accelerator in
        # a transient bad state that clears on the next dispatch
        import time
        time.sleep(2.0)
        res = run_bass_kernel_spmd(nc, in_maps, list(range(NCORES)))
    out = np.empty((N, OUT), dtype=np.float32)
    for c in range(NCORES):
        out[assign[c]] = res.results[c]["out"][pos0_all[c]]
    return out


# revision 10
# speedup vs baseline: 1.6504x; 1.6504x over previous
"""BankedLinear (MoE-style banked linear) Trainium2 Bass kernel.

Math: out[n] = sum_k bank_weights[n,k] * (tensor[n] @ W[sel[n,k]] + bias[sel[n,k]])
Shapes: tensor [8192,128] f32, bank_weights [8192,2] f32, bank_selections [8192,2] int,
        weights [64,128,128] f32, bias [64,128] f32 -> out [8192,128] f32.

Strategy (data parallel over tokens, weights replicated, fp16 compute):
  - 8 cores x 1024 tokens, greedily assigned so per-bank pair counts are
    balanced across cores (SPMD: one program, shared per-bank capacity plan).
  - Two sorted passes per core. Pass A processes each token's k=0 pair with
    tokens sorted by sel[:,0]; pass B processes k=1 pairs sorted by sel[:,1].
    The host pre-builds x^T column tiles in each sorted order with the
    bank_weight folded in (column s = bw[pair s] * x[token s], fp16), so a
    single fp16 matmul per bank segment computes bw*x@W. The bias term is
    added per 512-column psum chunk with one 64-contraction matmul against a
    one-hot routing matrix sm[b, s] = bw[s]*[bank(s)==b] (the aux tensor
    packs sm0/sm1 on partition halves plus a duplicated bias block).
  - psum -> fp16 SBUF evict, PE-transpose back to row layout, evict to f32
    rows. Pass A rows land in slot order = output order: contiguous DMA to
    out. Pass B rows are combined with a single SWDGE dma_scatter_add
    (out[idx[j]] += rows1[j]) whose per-core index tensor maps each k=1 slot
    to its token's k=0 slot. Pad rows are exactly zero and are pointed at
    pass-A pad slots (never at a real row: on hardware the DRAM
    read-modify-write of a pad's +0 add can race a real add).
  - Host unshards: out[assign[c][i]] = dev_out[c][pos0[c][i]].
"""

import numpy as np

N, K, IN, OUT, NUM_BANKS = 8192, 2, 128, 128, 64
NCORES = 8
NLOC = N // NCORES
P = 128
PSUM_FREE = 512


def _routing_plan(sel_all):
    """Greedy token->core assignment balancing per-bank counts for both the
    k=0 and k=1 pair distributions. Returns (assign, caps0, offs0, caps1,
    offs1, Cap) with a shared capacity plan (SPMD) and equal Cap per pass."""
    sel_all = np.asarray(sel_all).astype(np.int64)
    g0 = np.bincount(sel_all[:, 0], minlength=NUM_BANKS)
    g1 = np.bincount(sel_all[:, 1], minlength=NUM_BANKS)
    ideal0 = (g0 + NCORES - 1) // NCORES
    ideal1 = (g1 + NCORES - 1) // NCORES
    c0 = np.zeros((NCORES, NUM_BANKS), dtype=np.int64)
    c1 = np.zeros((NCORES, NUM_BANKS), dtype=np.int64)
    fill = np.zeros(NCORES, dtype=np.int64)
    assign_lists = [[] for _ in range(NCORES)]
    for n in range(N):
        b0, b1 = int(sel_all[n, 0]), int(sel_all[n, 1])
        best, best_key = -1, None
        for c in range(NCORES):
            if fill[c] >= NLOC:
                continue
            over = max(0, c0[c, b0] + 1 - ideal0[b0]) + \
                max(0, c1[c, b1] + 1 - ideal1[b1])
            key = (over, c0[c, b0] + c1[c, b1], fill[c])
            if best < 0 or key < best_key:
                best, best_key = c, key
        c0[best, b0] += 1
        c1[best, b1] += 1
        fill[best] += 1
        assign_lists[best].append(n)
    assign = np.array(assign_lists, dtype=np.int64)

    caps0 = c0.max(axis=0).astype(np.int64)
    caps1 = c1.max(axis=0).astype(np.int64)
    Cap = -(-max(int(caps0.sum()), int(caps1.sum())) // P) * P

    def pad_to(caps, target):
        pad = target - int(caps.sum())
        for i in range(pad):
            caps[i % NUM_BANKS] += 1
        offs = np.concatenate([[0], np.cumsum(caps)[:-1]]).astype(np.int64)
        return caps, offs

    caps0, offs0 = pad_to(caps0, Cap)
    caps1, offs1 = pad_to(caps1, Cap)
    return assign, caps0, offs0, caps1, offs1, Cap


def _wrap_idx(flat_idx):
    """Wrap a flat int16 index list into the [128, n//16] SWDGE layout."""
    n = flat_idx.shape[0]
    assert n % 16 == 0
    w = flat_idx.reshape(n // 16, 16).T.astype(np.int16)
    return np.tile(w, (8, 1))


def _segments(caps, offs):
    """Per-bank psum column segments split at PSUM_FREE boundaries.
    Returns list of (bank, col_start, width) with width>0."""
    segs = []
    for b in range(NUM_BANKS):
        cb, ob = int(caps[b]), int(offs[b])
        while cb > 0:
            room = PSUM_FREE - (ob % PSUM_FREE)
            w = min(cb, room)
            segs.append((b, ob, w))
            ob += w
            cb -= w
    return segs


def _build_program(caps0, offs0, caps1, offs1, Cap):
    import concourse.bacc as bacc
    import concourse.tile as tile
    from concourse import mybir, library_config
    from concourse.masks import make_identity
    from concourse.tile import add_dep_helper

    f32 = mybir.dt.float32
    f16 = mybir.dt.float16
    i16 = mybir.dt.int16

    nblk = Cap // P
    nch = (Cap + PSUM_FREE - 1) // PSUM_FREE

    nc = bacc.Bacc(None, target_bir_lowering=False, debug=False)

    x0_d = nc.declare_dram_parameter("x0", [P, Cap], f16, isOutput=False)
    x1_d = nc.declare_dram_parameter("x1", [P, Cap], f16, isOutput=False)
    w_d = nc.declare_dram_parameter("wts", [P, NUM_BANKS * OUT], f16,
                                    isOutput=False)
    aux_d = nc.declare_dram_parameter("aux", [P, Cap + P], f16, isOutput=False)
    idx_d = nc.declare_dram_parameter("sidx", [P, Cap // 16], i16,
                                      isOutput=False)
    out_d = nc.declare_dram_parameter("out", [Cap, OUT], f32, isOutput=True)

    segs0 = _segments(caps0, offs0)
    segs1 = _segments(caps1, offs1)

    # weight DMA split across three queues, in bank order
    wsplit = (22, 22, 20)
    wb = [0, wsplit[0], wsplit[0] + wsplit[1], NUM_BANKS]

    with tile.TileContext(nc) as tc:
        with (
            tc.tile_pool(name="const", bufs=1) as cpool,
            tc.tile_pool(name="big", bufs=1) as bigpool,
            tc.tile_pool(name="psA", bufs=1, space="PSUM") as psA,
            tc.tile_pool(name="psB", bufs=1, space="PSUM") as psB,
            tc.tile_pool(name="psT", bufs=2, space="PSUM") as psT,
        ):
            ident = cpool.tile([P, P], f16)
            make_identity(nc, ident[:])
            libload = nc.gpsimd.load_library(library_config.mlp)

            # input DMAs
            xs0 = cpool.tile([P, Cap], f16)
            nc.sync.dma_start(out=xs0[:], in_=x0_d.ap())
            aux_sb = cpool.tile([P, Cap + P], f16)
            nc.sync.dma_start(out=aux_sb[:], in_=aux_d.ap())
            xs1 = cpool.tile([P, Cap], f16)
            nc.sync.dma_start(out=xs1[:], in_=x1_d.ap())
            idx_sb = cpool.tile([P, Cap // 16], i16)
            nc.sync.dma_start(out=idx_sb[:], in_=idx_d.ap())

            w_sb = cpool.tile([P, NUM_BANKS * OUT], f16)
            for si, eng in zip(range(3), (nc.scalar, nc.scalar, nc.gpsimd)):
                lo, hi = wb[si] * OUT, wb[si + 1] * OUT
                eng.dma_start(out=w_sb[:, lo:hi], in_=w_d[:, lo:hi])

            # per-bank matmuls into chunked psum tiles (both passes)
            pA, pB = [], []
            for i in range(nch):
                pt = psA.tile([P, PSUM_FREE], f32, tag=f"pA{i}", name=f"pA{i}")
                pA.append(pt)
            for i in range(nch):
                pt = psB.tile([P, PSUM_FREE], f32, tag=f"pB{i}", name=f"pB{i}")
                pB.append(pt)

            def do_pass(segs, ptiles, xs, pbase):
                # one psum accumulation group per chunk: start=True only on
                # the chunk's first segment (lazy-zeroes the whole 2KB
                # region), stop=True only on the closing bias matmul
                seen = set()
                for (b, ob, wseg) in segs:
                    ch, co = ob // PSUM_FREE, ob % PSUM_FREE
                    first = ch not in seen
                    seen.add(ch)
                    nc.tensor.matmul(out=ptiles[ch][:, co:co + wseg],
                                     lhsT=w_sb[:, b * OUT:(b + 1) * OUT],
                                     rhs=xs[:, ob:ob + wseg],
                                     start=first, stop=False)
                for ch in range(nch):
                    lo = ch * PSUM_FREE
                    wch = min(PSUM_FREE, Cap - lo)
                    nc.tensor.matmul(
                        out=ptiles[ch][:, :wch],
                        lhsT=aux_sb[pbase:pbase + NUM_BANKS, Cap:Cap + P],
                        rhs=aux_sb[pbase:pbase + NUM_BANKS, lo:lo + wch],
                        start=False, stop=True)

            do_pass(segs0, pA, xs0, 0)
            do_pass(segs1, pB, xs1, NUM_BANKS)

            # evict psum chunks -> fp16, transpose -> f32 rows
            y0T = bigpool.tile([P, Cap], f16, tag="y0T")
            y1T = bigpool.tile([P, Cap], f16, tag="y1T")
            rows0 = bigpool.tile([P, nblk, OUT], f32, tag="rows0")
            rows1 = bigpool.tile([P, nblk, OUT], f32, tag="rows1")

            flip = 0
            for (yT, rows, ptiles) in ((y0T, rows0, pA), (y1T, rows1, pB)):
                for ch in range(nch):
                    lo = ch * PSUM_FREE
                    wch = min(PSUM_FREE, Cap - lo)
                    h = wch // 2
                    eng0 = nc.vector.tensor_copy if flip == 0 else nc.scalar.copy
                    eng1 = nc.scalar.copy if flip == 0 else nc.vector.tensor_copy
                    if h > 0:
                        eng0(yT[:, lo:lo + h], ptiles[ch][:, :h])
                        eng1(yT[:, lo + h:lo + wch], ptiles[ch][:, h:wch])
                    else:
                        eng0(yT[:, lo:lo + wch], ptiles[ch][:, :wch])
                    flip ^= 1
                for t in range(nblk):
                    ptt = psT.tile([P, 4 * P], f16, tag="ptt")
                    nc.tensor.transpose(out=ptt[:, :P],
                                        in_=yT[:, t * P:(t + 1) * P],
                                        identity=ident[:])
                    if t % 2 == 0:
                        nc.vector.tensor_copy(rows[:, t, :], ptt[:, :P])
                    else:
                        nc.scalar.copy(rows[:, t, :], ptt[:, :P])

            # pass A rows -> contiguous out; pass B rows -> scatter-add
            hb = nblk // 2
            nc.sync.dma_start(
                out=out_d[:hb * P].rearrange("(t p) o -> p t o", p=P),
                in_=rows0[:, :hb, :])
            nc.sync.dma_start(
                out=out_d[hb * P:].rearrange("(t p) o -> p t o", p=P),
                in_=rows0[:, hb:, :])
            sa = nc.gpsimd.dma_scatter_add(
                out_ap=out_d.ap(),
                in_ap=rows1[:, :, :],
                idxs_ap=idx_sb[:],
                num_idxs=Cap,
                num_idxs_reg=Cap,
                elem_size=OUT,
                single_packet=Cap <= 1024,
            )
            add_dep_helper(sa.ins, libload.ins, sync=False,
                           reason="scatter-add needs mlp gpsimd library")

    return nc


def _make_in_maps(tensor, bank_weights, bank_selections, weights, bias,
                  assign, caps0, offs0, caps1, offs1, Cap):
    tensor = np.ascontiguousarray(tensor, dtype=np.float32)
    bank_weights = np.ascontiguousarray(bank_weights, dtype=np.float32)
    sel_all = np.asarray(bank_selections).astype(np.int64)
    w16 = np.ascontiguousarray(
        np.asarray(weights, dtype=np.float32).transpose(1, 0, 2)
        .reshape(IN, NUM_BANKS * OUT)).astype(np.float16)
    bias16 = np.ascontiguousarray(bias, dtype=np.float32).astype(np.float16)

    in_maps = []
    pos0_all = []
    for c in range(NCORES):
        toks = assign[c]
        sel = sel_all[toks]                      # [NLOC, K]
        bw = bank_weights[toks]                  # [NLOC, K]
        x = tensor[toks]                         # [NLOC, IN]

        def lay(k, offs):
            slot = np.zeros(NLOC, dtype=np.int64)
            fillb = offs.copy()
            for i in range(NLOC):
                b = sel[i, k]
                slot[i] = fillb[b]
                fillb[b] += 1
            xbw = np.zeros((Cap, IN), dtype=np.float32)
            xbw[slot] = x * bw[:, k:k + 1]
            sm = np.zeros((NUM_BANKS, Cap), dtype=np.float32)
            sm[sel[:, k], slot] = bw[:, k]
            return slot, np.ascontiguousarray(xbw.T).astype(np.float16), \
                sm.astype(np.float16)

        slot0, x0, sm0 = lay(0, offs0)
        slot1, x1, sm1 = lay(1, offs1)
        pos0_all.append(slot0)

        aux = np.zeros((P, Cap + P), dtype=np.float16)
        aux[:NUM_BANKS, :Cap] = sm0
        aux[NUM_BANKS:, :Cap] = sm1
        aux[:NUM_BANKS, Cap:] = bias16
        aux[NUM_BANKS:, Cap:] = bias16

        # scatter indices: k=1 slot j -> that token's k=0 slot. Pad rows are
        # all-zero but their scatter-add is a DRAM read-modify-write that can
        # race a real add to the same row on hardware — point them at pass-A
        # pad slots (rows no token reads) instead of a shared real row.
        pad0 = np.setdiff1d(np.arange(Cap, dtype=np.int64), slot0)
        assert pad0.size > 0
        reps = (Cap + pad0.size - 1) // pad0.size
        sidx = np.tile(pad0, reps)[:Cap]
        sidx[slot1] = slot0
        in_maps.append({
            "x0": x0,
            "x1": x1,
            "wts": w16,
            "aux": aux,
            "sidx": _wrap_idx(sidx.astype(np.int16)),
        })
    return in_maps, pos0_all


def kernel(tensor, bank_weights, bank_selections, weights, bias):
    tensor = np.asarray(tensor)
    bank_weights = np.asarray(bank_weights)
    bank_selections = np.asarray(bank_selections)
    weights = np.asarray(weights)
    bias = np.asarray(bias)

    assign, caps0, offs0, caps1, offs1, Cap = _routing_plan(bank_selections)
    nc = _build_program(caps0, offs0, caps1, offs1, Cap)
    in_maps, pos0_all = _make_in_maps(
        tensor, bank_weights, bank_selections, weights, bias,
        assign, caps0, offs0, caps1, offs1, Cap)

    nc.finalize()
    from concourse.bass_utils import run_bass_kernel_spmd
    try:
        res = run_bass_kernel_spmd(nc, in_maps, list(range(NCORES)))
    except Exception:
        # one retry: a previous crashed session can leave the accelerator in
        # a transient bad state that clears on the next dispatch
        import time
        time.sleep(2.0)
        res = run_bass_kernel_spmd(nc, in_maps, list(range(NCORES)))
    out = np.empty((N, OUT), dtype=np.float32)
    for c in range(NCORES):
        out[assign[c]] = res.results[c]["out"][pos0_all[c]]
    return out
